# revision 40
# baseline (speedup 1.0000x reference)
import sys

sys.path.insert(0, "/opt/trn_rl_repo")

import numpy as np

N = 1024
NCORES = 8
DV = 64  # Chebyshev expansion order per axis
GFIT = 128  # fit grid size
# Symmetric-upper gram blocks (m = 128-row block of output, n = 512-col block).
# Each block only computes columns >= its row-block start (lower part comes
# from the host-side mirror), i.e. within-block columns [LO, 512).
# n=0 blocks first (they only need the first half of K), narrow blocks last
# within each half so the final evac/DMA are the smallest.
BLOCKS = [(m, 0) for m in range(4)] + [(m, 1) for m in range(8)]
LOS = [max(0, 128 * m - 512 * n) for (m, n) in BLOCKS]
NBLK = len(BLOCKS)  # 12
WID = [512 - lo for lo in LOS]
OFF = [0]
for _w in WID[:-1]:
    OFF.append(OFF[-1] + _w)
TOTW = OFF[-1] + WID[-1]  # 4608
# output DMA groups: blocks [0,4), [4,8), [8,12)
GRP = [(0, 4), (4, 8), (8, 12)]
GSPAN = [(OFF[a], OFF[b - 1] + WID[b - 1]) for a, b in GRP]

_BUILD_CACHE = {}
LAST_RESULT = None


def _build():
    import concourse.bass as bass
    from concourse import mybir

    F32 = mybir.dt.float32
    BF16 = mybir.dt.bfloat16
    U8 = mybir.dt.uint8

    nc = bass.Bass("TRN2", target_bir_lowering=False, debug=False, num_devices=8)

    TBA_d = nc.dram_tensor("TBA", (DV, N + 128), BF16, kind="ExternalInput")
    MS_d = nc.dram_tensor("MS", (128, N), U8, kind="ExternalInput")
    o_d = [
        nc.dram_tensor(
            f"o{g}", (128, GSPAN[g][1] - GSPAN[g][0]), BF16, kind="ExternalOutput"
        )
        for g in range(3)
    ]
    u_d = nc.dram_tensor("u", (128, N), BF16, kind="ExternalOutput")

    TBA_s = nc.alloc_sbuf_tensor("TBA_s", [DV, N + 128], BF16)
    K_s = nc.alloc_sbuf_tensor("K_s", [128, N], BF16)
    MS_s = nc.alloc_sbuf_tensor("MS_s", [128, N], U8)
    o_s = nc.alloc_sbuf_tensor("o_s", [128, TOTW], BF16)
    scr = nc.alloc_sbuf_tensor("scr", [128, 2], F32)

    pv = nc.alloc_psum_tensor("pv", [128, 1024], F32)  # 2 banks: V halves
    pg = [nc.alloc_psum_tensor(f"pg{i}", [128, 1024], F32) for i in range(3)]

    def mk_waiter(engine):
        seen = {}

        def w(sem, val):
            if seen.get(id(sem), 0) < val:
                engine.wait_ge(sem, val)
                seen[id(sem)] = val

        return w

    with (
        nc.Block() as block,
        nc.semaphore("dmaA") as dmaA,
        nc.semaphore("dmaA2") as dmaA2,
        nc.semaphore("dmaB") as dmaB,
        nc.semaphore("msem") as msem,
        nc.semaphore("pes") as pes,
        nc.semaphore("dves") as dves,
        nc.semaphore("evA") as evA,
        nc.semaphore("evB") as evB,
        nc.semaphore("odma") as odma,
    ):

        @block.sync
        def _(sync):
            w = mk_waiter(sync)
            # TBA layout: [A (0:128) | TB (128:1152)]
            sync.dma_start(
                out=TBA_s.ap()[:, 0:640], in_=TBA_d[:, 0:640]
            ).then_inc(dmaA, 16)
            sync.dma_start(
                out=TBA_s.ap()[:, 640 : N + 128], in_=TBA_d[:, 640 : N + 128]
            ).then_inc(dmaA2, 16)
            w(dves, 2)
            sync.dma_start(out=u_d[:], in_=K_s.ap()).then_inc(odma, 16)
            for g in range(3):
                w(evA, 2 * (g + 1))
                w(evB, 2 * (g + 1))
                sync.dma_start(
                    out=o_d[g][:],
                    in_=o_s.ap()[:, GSPAN[g][0] : GSPAN[g][1]],
                ).then_inc(odma, 16)

        @block.gpsimd
        def _(gp):
            gp.memset(K_s.ap(), 0.0).then_inc(msem, 1)

        @block.tensor
        def _(tensor):
            w = mk_waiter(tensor)
            w(dmaA, 16)
            tensor.matmul(
                pv.ap()[:, 0:512],
                TBA_s.ap()[:, 0:128],
                TBA_s.ap()[:, 128:640],
                start=True,
                stop=True,
            ).then_inc(pes, 1)
            w(dmaA2, 16)
            tensor.matmul(
                pv.ap()[:, 512:1024],
                TBA_s.ap()[:, 0:128],
                TBA_s.ap()[:, 640:1152],
                start=True,
                stop=True,
            ).then_inc(pes, 1)
            # blocks 0..9 use pg slots 0..5 (reused); blocks 10,11 use the pv
            # halves freed by the predicated copies
            for k, (m, n) in enumerate(BLOCKS):
                w(dves, 1 if n == 0 else 2)
                if 6 <= k < 10:
                    j = k - 6  # evac of block j freed slot j
                    if j % 2 == 0:
                        w(evA, j // 2 + 1)
                    else:
                        w(evB, j // 2 + 1)
                lo = LOS[k]
                if k < 10:
                    s = k % 6
                    dst = pg[s // 2].ap()[
                        :, 512 * (s % 2) + lo : 512 * (s % 2) + 512
                    ]
                else:
                    h = k - 10
                    dst = pv.ap()[:, 512 * h + lo : 512 * h + 512]
                tensor.matmul(
                    dst,
                    K_s.ap()[:, 128 * m : 128 * m + 128],
                    K_s.ap()[:, 512 * n + lo : 512 * n + 512],
                    start=True,
                    stop=True,
                ).then_inc(pes, 1)

        @block.scalar
        def _(scalar):
            w = mk_waiter(scalar)
            scalar.dma_start(out=MS_s.ap(), in_=MS_d[:]).then_inc(dmaB, 16)
            # preload the Copy activation table while DMAs run
            scalar.copy(scr.ap()[:, 0:1], scr.ap()[:, 1:2])
            for k in (0, 2, 4, 6, 8, 10):
                w(pes, 2 + k + 1)
                lo = LOS[k]
                if k < 10:
                    s = k % 6
                    src = pg[s // 2].ap()[
                        :, 512 * (s % 2) + lo : 512 * (s % 2) + 512
                    ]
                else:
                    h = k - 10
                    src = pv.ap()[:, 512 * h + lo : 512 * h + 512]
                scalar.copy(
                    o_s.ap()[:, OFF[k] : OFF[k] + WID[k]], src
                ).then_inc(evA, 1)

        @block.vector
        def _(vector):
            w = mk_waiter(vector)
            w(msem, 1)
            w(dmaB, 16)
            w(pes, 1)
            vector.copy_predicated(
                K_s.ap()[:, 0:512], MS_s.ap()[:, 0:512], pv.ap()[:, 0:512]
            ).then_inc(dves, 1)
            w(pes, 2)
            vector.copy_predicated(
                K_s.ap()[:, 512:1024], MS_s.ap()[:, 512:1024], pv.ap()[:, 512:1024]
            ).then_inc(dves, 1)
            for k in (1, 3, 5, 7, 9, 11):
                w(pes, 2 + k + 1)
                lo = LOS[k]
                if k < 10:
                    s = k % 6
                    src = pg[s // 2].ap()[
                        :, 512 * (s % 2) + lo : 512 * (s % 2) + 512
                    ]
                else:
                    h = k - 10
                    src = pv.ap()[:, 512 * h + lo : 512 * h + 512]
                vector.tensor_copy(
                    o_s.ap()[:, OFF[k] : OFF[k] + WID[k]], src
                ).then_inc(evB, 1)

    return nc


def _cheb_basis(vals, Dp):
    z = 2.0 * vals - 1.0
    B = np.zeros((len(vals), Dp), np.float64)
    B[:, 0] = 1.0
    if Dp > 1:
        B[:, 1] = z
    for k in range(2, Dp):
        B[:, k] = 2 * z * B[:, k - 1] - B[:, k - 2]
    return B


def _fit_coeffs(W1, b1, W2, b2, W3, b3):
    # 2D Chebyshev-interpolation coefficients of the full MLP scalar output
    # v(s, t) on [0,1]^2, via tensor Chebyshev grid + DCT.
    G = GFIT
    k = np.arange(G)
    t = np.cos((2 * k + 1) * np.pi / (2 * G))
    s01 = (t + 1.0) / 2.0
    S, T = np.meshgrid(s01, s01, indexing="ij")
    u = (
        W1[:, 0][:, None, None] * S[None]
        + W1[:, 1][:, None, None] * T[None]
        + b1[:, None, None]
    )
    h2 = np.tensordot(W2, np.tanh(u), axes=(1, 0)) + b2[:, None, None]
    F = np.tensordot(W3[0], np.maximum(h2, 0.0), axes=(0, 0)) + b3[0]
    # Chebyshev coefficients via plain cosine transform (DCT-II equivalent)
    theta = (2 * k + 1)[None, :] * np.arange(G)[:, None] * (np.pi / (2 * G))
    W = np.cos(theta) * (2.0 / G)  # [k, i]
    W[0, :] /= 2.0
    C = W @ F @ W.T
    return C[:DV, :DV]


def kernel(x, W1, b1, W2, b2, W3, b3, sigma, _trace=False):
    from concourse.bass_utils import run_bass_kernel_spmd
    import ml_dtypes

    bf16 = ml_dtypes.bfloat16

    x = np.asarray(x, np.float64).reshape(N)
    W1 = np.asarray(W1, np.float64)
    b1 = np.asarray(b1, np.float64).reshape(128)
    W2 = np.asarray(W2, np.float64)
    b2 = np.asarray(b2, np.float64).reshape(32)
    W3 = np.asarray(W3, np.float64).reshape(1, 32)
    b3 = np.asarray(b3, np.float64).reshape(1)
    sig = float(np.asarray(sigma, np.float64).reshape(-1)[0])

    if "nc" not in _BUILD_CACHE:
        _BUILD_CACHE["nc"] = _build()
    nc = _BUILD_CACHE["nc"]

    C = _fit_coeffs(W1, b1, W2, b2, W3, b3)  # [DV, DV]
    Sb = _cheb_basis(x, DV)  # [N, DV]

    jcols = np.arange(N)
    in_maps = []
    ips = []
    for c in range(NCORES):
        ip = 8 * np.arange(128) + c
        ips.append(ip)
        TBA = np.zeros((DV, N + 128), bf16)
        TBA[:, :128] = (C.T @ Sb[ip].T).astype(bf16)
        TBA[:, 128:] = Sb.T.astype(bf16)
        MS = (jcols[None, :] > ip[:, None]).astype(np.uint8)
        in_maps.append({"TBA": TBA, "MS": MS})

    res = run_bass_kernel_spmd(
        nc, in_maps, core_ids=list(range(NCORES)), trace=_trace
    )
    global LAST_RESULT
    LAST_RESULT = res

    # Sum bf16 partial gram blocks (M^T M) across cores, mirror, then add the
    # identity-row cross terms: out = sigma^2 (M^T M + U + U^T + I).
    P = np.zeros((N, N), np.float32)
    for k, (m, n) in enumerate(BLOCKS):
        g = k // 4
        lo = LOS[k]
        st = OFF[k] - GSPAN[g][0]
        acc = np.zeros((128, WID[k]), np.float32)
        for c in range(NCORES):
            acc += res.results[c][f"o{g}"][:, st : st + WID[k]].astype(
                np.float32
            )
        P[128 * m : 128 * (m + 1), 512 * n + lo : 512 * (n + 1)] = acc
    # mirror everything left of each row-block's diagonal start
    for m in range(1, 8):
        P[128 * m : 128 * (m + 1), 0 : 128 * m] = P[
            0 : 128 * m, 128 * m : 128 * (m + 1)
        ].T
    U = np.zeros((N, N), np.float32)
    for c in range(NCORES):
        U[ips[c], :] = res.results[c]["u"].astype(np.float32)
    P += U
    P += U.T
    P[np.arange(N), np.arange(N)] += 1.0
    return (np.float32(sig) * np.float32(sig)) * P


# revision 47
# speedup vs baseline: 1.0023x; 1.0023x over previous
import sys

sys.path.insert(0, "/opt/trn_rl_repo")

import numpy as np

N = 1024
NCORES = 8
DV = 64  # Chebyshev expansion order per axis
GFIT = 128  # fit grid size
# Symmetric-upper gram blocks (m = 128-row block of output, n = 512-col block).
# Each block only computes columns >= its row-block start (lower part comes
# from the host-side mirror), i.e. within-block columns [LO, 512).
# n=0 blocks first (they only need the first half of K), narrow blocks last
# within each half so the final evac/DMA are the smallest.
BLOCKS = [(m, 0) for m in range(4)] + [(m, 1) for m in range(8)]
LOS = [max(0, 128 * m - 512 * n) for (m, n) in BLOCKS]
NBLK = len(BLOCKS)  # 12
WID = [512 - lo for lo in LOS]
OFF = [0]
for _w in WID[:-1]:
    OFF.append(OFF[-1] + _w)
TOTW = OFF[-1] + WID[-1]  # 4608
# output DMA groups: blocks [0,4), [4,8), [8,12)
GRP = [(0, 4), (4, 8), (8, 12)]
GSPAN = [(OFF[a], OFF[b - 1] + WID[b - 1]) for a, b in GRP]



# slot map: block -> (pg tensor halves) keeping the original 3x[128,1024]
# layout; slots: 0..5 = pg halves; b7/b8 = pv halves; b9..b11 reuse 1,2,3
_SLOTS = {0: 0, 1: 1, 2: 2, 3: 3, 4: 4, 5: 5, 6: 0, 9: 1, 10: 2, 11: 3}


def SLOT_AP(pg, pv, k, lo):
    if k in (7, 8):
        h = k - 7
        return pv.ap()[:, 512 * h + lo : 512 * h + 512]
    s = _SLOTS[k]
    return pg[s // 2].ap()[:, 512 * (s % 2) + lo : 512 * (s % 2) + 512]

_BUILD_CACHE = {}
LAST_RESULT = None


def _build():
    import concourse.bass as bass
    from concourse import mybir

    F32 = mybir.dt.float32
    BF16 = mybir.dt.bfloat16
    U8 = mybir.dt.uint8

    nc = bass.Bass("TRN2", target_bir_lowering=False, debug=False, num_devices=8)

    TBA_d = nc.dram_tensor("TBA", (DV, N + 128), BF16, kind="ExternalInput")
    MS_d = nc.dram_tensor("MS", (128, N), U8, kind="ExternalInput")
    o_d = [
        nc.dram_tensor(
            f"o{g}", (128, GSPAN[g][1] - GSPAN[g][0]), BF16, kind="ExternalOutput"
        )
        for g in range(3)
    ]
    u_d = nc.dram_tensor("u", (128, N), BF16, kind="ExternalOutput")

    TBA_s = nc.alloc_sbuf_tensor("TBA_s", [DV, N + 128], BF16)
    K_s = nc.alloc_sbuf_tensor("K_s", [128, N], BF16)
    MS_s = nc.alloc_sbuf_tensor("MS_s", [128, N], U8)
    o_s = nc.alloc_sbuf_tensor("o_s", [128, TOTW], BF16)
    scr = nc.alloc_sbuf_tensor("scr", [128, 2], F32)

    pv = nc.alloc_psum_tensor("pv", [128, 1024], F32)  # 2 banks: V halves
    pg = [nc.alloc_psum_tensor(f"pg{i}", [128, 1024], F32) for i in range(3)]

    def mk_waiter(engine):
        seen = {}

        def w(sem, val):
            if seen.get(id(sem), 0) < val:
                engine.wait_ge(sem, val)
                seen[id(sem)] = val

        return w

    with (
        nc.Block() as block,
        nc.semaphore("dmaA") as dmaA,
        nc.semaphore("dmaA2") as dmaA2,
        nc.semaphore("dmaB") as dmaB,
        nc.semaphore("msem") as msem,
        nc.semaphore("pes") as pes,
        nc.semaphore("dves") as dves,
        nc.semaphore("evA") as evA,
        nc.semaphore("evB") as evB,
        nc.semaphore("odma") as odma,
    ):

        @block.sync
        def _(sync):
            w = mk_waiter(sync)
            # TBA layout: [A (0:128) | TB (128:1152)]
            sync.dma_start(
                out=TBA_s.ap()[:, 0:640], in_=TBA_d[:, 0:640]
            ).then_inc(dmaA, 16)
            sync.dma_start(
                out=TBA_s.ap()[:, 640 : N + 128], in_=TBA_d[:, 640 : N + 128]
            ).then_inc(dmaA2, 16)
            w(dves, 2)
            sync.dma_start(out=u_d[:], in_=K_s.ap()).then_inc(odma, 16)
            for g in range(3):
                w(evA, 2 * (g + 1))
                w(evB, 2 * (g + 1))
                sync.dma_start(
                    out=o_d[g][:],
                    in_=o_s.ap()[:, GSPAN[g][0] : GSPAN[g][1]],
                ).then_inc(odma, 16)

        @block.gpsimd
        def _(gp):
            gp.memset(K_s.ap(), 0.0).then_inc(msem, 1)

        @block.tensor
        def _(tensor):
            w = mk_waiter(tensor)
            w(dmaA, 16)
            tensor.matmul(
                pv.ap()[:, 0:512],
                TBA_s.ap()[:, 0:128],
                TBA_s.ap()[:, 128:640],
                start=True,
                stop=True,
            ).then_inc(pes, 1)
            w(dmaA2, 16)
            tensor.matmul(
                pv.ap()[:, 512:1024],
                TBA_s.ap()[:, 0:128],
                TBA_s.ap()[:, 640:1152],
                start=True,
                stop=True,
            ).then_inc(pes, 1)
            # blocks 0-6 use pg slots 0-6 fresh; b7/b8 take the pv halves
            # freed by the predicated copies (already waited via dves>=2);
            # b9/b10/b11 reuse slots 1,2,3 whose evacs finish much earlier.
            for k, (m, n) in enumerate(BLOCKS):
                w(dves, 1 if n == 0 else 2)
                if k == 6:
                    w(evA, 1)  # slot 0 freed by b0's evac
                elif k == 9:
                    w(evB, 1)  # slot 1 freed by b1's evac
                elif k == 10:
                    w(evA, 2)  # slot 2 freed by b2's evac
                elif k == 11:
                    w(evB, 2)  # slot 3 freed by b3's evac
                lo = LOS[k]
                dst = SLOT_AP(pg, pv, k, lo)
                tensor.matmul(
                    dst,
                    K_s.ap()[:, 128 * m : 128 * m + 128],
                    K_s.ap()[:, 512 * n + lo : 512 * n + 512],
                    start=True,
                    stop=True,
                ).then_inc(pes, 1)

        @block.scalar
        def _(scalar):
            w = mk_waiter(scalar)
            scalar.dma_start(out=MS_s.ap(), in_=MS_d[:]).then_inc(dmaB, 16)
            # preload the Copy activation table while DMAs run
            scalar.copy(scr.ap()[:, 0:1], scr.ap()[:, 1:2])
            for k in (0, 2, 4, 6, 8, 10):
                w(pes, 2 + k + 1)
                lo = LOS[k]
                src = SLOT_AP(pg, pv, k, lo)
                scalar.copy(
                    o_s.ap()[:, OFF[k] : OFF[k] + WID[k]], src
                ).then_inc(evA, 1)

        @block.vector
        def _(vector):
            w = mk_waiter(vector)
            w(msem, 1)
            w(dmaB, 16)
            w(pes, 1)
            vector.copy_predicated(
                K_s.ap()[:, 0:512], MS_s.ap()[:, 0:512], pv.ap()[:, 0:512]
            ).then_inc(dves, 1)
            w(pes, 2)
            vector.copy_predicated(
                K_s.ap()[:, 512:1024], MS_s.ap()[:, 512:1024], pv.ap()[:, 512:1024]
            ).then_inc(dves, 1)
            for k in (1, 3, 5, 7, 9, 11):
                w(pes, 2 + k + 1)
                lo = LOS[k]
                src = SLOT_AP(pg, pv, k, lo)
                vector.tensor_copy(
                    o_s.ap()[:, OFF[k] : OFF[k] + WID[k]], src
                ).then_inc(evB, 1)

    return nc


def _cheb_basis(vals, Dp):
    z = 2.0 * vals - 1.0
    B = np.zeros((len(vals), Dp), np.float64)
    B[:, 0] = 1.0
    if Dp > 1:
        B[:, 1] = z
    for k in range(2, Dp):
        B[:, k] = 2 * z * B[:, k - 1] - B[:, k - 2]
    return B


def _fit_coeffs(W1, b1, W2, b2, W3, b3):
    # 2D Chebyshev-interpolation coefficients of the full MLP scalar output
    # v(s, t) on [0,1]^2, via tensor Chebyshev grid + DCT.
    G = GFIT
    k = np.arange(G)
    t = np.cos((2 * k + 1) * np.pi / (2 * G))
    s01 = (t + 1.0) / 2.0
    S, T = np.meshgrid(s01, s01, indexing="ij")
    u = (
        W1[:, 0][:, None, None] * S[None]
        + W1[:, 1][:, None, None] * T[None]
        + b1[:, None, None]
    )
    h2 = np.tensordot(W2, np.tanh(u), axes=(1, 0)) + b2[:, None, None]
    F = np.tensordot(W3[0], np.maximum(h2, 0.0), axes=(0, 0)) + b3[0]
    try:
        from scipy.fft import dct
    except ImportError:  # pragma: no cover
        from scipy.fftpack import dct
    C = dct(dct(F, type=2, axis=-1), type=2, axis=-2) / (G * G)
    C[0, :] /= 2.0
    C[:, 0] /= 2.0
    return C[:DV, :DV]


def kernel(x, W1, b1, W2, b2, W3, b3, sigma, _trace=False):
    from concourse.bass_utils import run_bass_kernel_spmd
    import ml_dtypes

    bf16 = ml_dtypes.bfloat16

    x = np.asarray(x, np.float64).reshape(N)
    W1 = np.asarray(W1, np.float64)
    b1 = np.asarray(b1, np.float64).reshape(128)
    W2 = np.asarray(W2, np.float64)
    b2 = np.asarray(b2, np.float64).reshape(32)
    W3 = np.asarray(W3, np.float64).reshape(1, 32)
    b3 = np.asarray(b3, np.float64).reshape(1)
    sig = float(np.asarray(sigma, np.float64).reshape(-1)[0])

    if "nc" not in _BUILD_CACHE:
        _BUILD_CACHE["nc"] = _build()
    nc = _BUILD_CACHE["nc"]

    C = _fit_coeffs(W1, b1, W2, b2, W3, b3)  # [DV, DV]
    Sb = _cheb_basis(x, DV)  # [N, DV]

    jcols = np.arange(N)
    in_maps = []
    ips = []
    for c in range(NCORES):
        ip = 8 * np.arange(128) + c
        ips.append(ip)
        TBA = np.zeros((DV, N + 128), bf16)
        TBA[:, :128] = (C.T @ Sb[ip].T).astype(bf16)
        TBA[:, 128:] = Sb.T.astype(bf16)
        MS = (jcols[None, :] > ip[:, None]).astype(np.uint8)
        in_maps.append({"TBA": TBA, "MS": MS})

    res = run_bass_kernel_spmd(
        nc, in_maps, core_ids=list(range(NCORES)), trace=_trace
    )
    global LAST_RESULT
    LAST_RESULT = res

    # Sum bf16 partial gram blocks (M^T M) across cores, mirror, then add the
    # identity-row cross terms: out = sigma^2 (M^T M + U + U^T + I).
    P = np.zeros((N, N), np.float32)
    for k, (m, n) in enumerate(BLOCKS):
        g = k // 4
        lo = LOS[k]
        st = OFF[k] - GSPAN[g][0]
        acc = np.zeros((128, WID[k]), np.float32)
        for c in range(NCORES):
            acc += res.results[c][f"o{g}"][:, st : st + WID[k]].astype(
                np.float32
            )
        P[128 * m : 128 * (m + 1), 512 * n + lo : 512 * (n + 1)] = acc
    # mirror everything left of each row-block's diagonal start
    for m in range(1, 8):
        P[128 * m : 128 * (m + 1), 0 : 128 * m] = P[
            0 : 128 * m, 128 * m : 128 * (m + 1)
        ].T
    U = np.zeros((N, N), np.float32)
    for c in range(NCORES):
        U[ips[c], :] = res.results[c]["u"].astype(np.float32)
    P += U
    P += U.T
    P[np.arange(N), np.arange(N)] += 1.0
    return (np.float32(sig) * np.float32(sig)) * P


# revision 48
# speedup vs baseline: 1.0050x; 1.0028x over previous
import sys

sys.path.insert(0, "/opt/trn_rl_repo")

import numpy as np

N = 1024
NCORES = 8
DV = 64  # Chebyshev expansion order per axis
GFIT = 128  # fit grid size
# Symmetric-upper gram blocks (m = 128-row block of output, n = 512-col block).
# Each block only computes columns >= its row-block start (lower part comes
# from the host-side mirror), i.e. within-block columns [LO, 512).
# n=0 blocks first (they only need the first half of K), narrow blocks last
# within each half so the final evac/DMA are the smallest.
BLOCKS = [(m, 0) for m in range(4)] + [(m, 1) for m in range(8)]
LOS = [max(0, 128 * m - 512 * n) for (m, n) in BLOCKS]
NBLK = len(BLOCKS)  # 12
WID = [512 - lo for lo in LOS]
OFF = [0]
for _w in WID[:-1]:
    OFF.append(OFF[-1] + _w)
TOTW = OFF[-1] + WID[-1]  # 4608
# output DMA groups: blocks [0,4), [4,8), [8,12)
GRP = [(0, 4), (4, 8), (8, 12)]
GSPAN = [(OFF[a], OFF[b - 1] + WID[b - 1]) for a, b in GRP]



# slot map: block -> (pg tensor halves) keeping the original 3x[128,1024]
# layout; slots: 0..5 = pg halves; b7/b8 = pv halves; b9..b11 reuse 1,2,3
_SLOTS = {0: 0, 1: 1, 2: 2, 3: 3, 4: 4, 5: 5, 6: 0, 9: 1, 10: 2, 11: 3}


def SLOT_AP(pg, pv, k, lo):
    if k in (7, 8):
        h = k - 7
        return pv.ap()[:, 512 * h + lo : 512 * h + 512]
    s = _SLOTS[k]
    return pg[s // 2].ap()[:, 512 * (s % 2) + lo : 512 * (s % 2) + 512]

_BUILD_CACHE = {}
LAST_RESULT = None


def _build():
    import concourse.bass as bass
    from concourse import mybir

    F32 = mybir.dt.float32
    BF16 = mybir.dt.bfloat16
    U8 = mybir.dt.uint8

    nc = bass.Bass("TRN2", target_bir_lowering=False, debug=False, num_devices=8)

    TBA_d = nc.dram_tensor("TBA", (DV, N + 128), BF16, kind="ExternalInput")
    MS_d = nc.dram_tensor("MS", (128, N), U8, kind="ExternalInput")
    o_d = [
        nc.dram_tensor(
            f"o{g}", (128, GSPAN[g][1] - GSPAN[g][0]), BF16, kind="ExternalOutput"
        )
        for g in range(3)
    ]
    u_d = nc.dram_tensor("u", (128, N), BF16, kind="ExternalOutput")

    TBA_s = nc.alloc_sbuf_tensor("TBA_s", [DV, N + 128], BF16)
    K_s = nc.alloc_sbuf_tensor("K_s", [128, N], BF16)
    MS_s = nc.alloc_sbuf_tensor("MS_s", [128, N], U8)
    o_s = nc.alloc_sbuf_tensor("o_s", [128, TOTW], BF16)
    scr = nc.alloc_sbuf_tensor("scr", [128, 2], F32)

    pv = nc.alloc_psum_tensor("pv", [128, 1024], F32)  # 2 banks: V halves
    pg = [nc.alloc_psum_tensor(f"pg{i}", [128, 1024], F32) for i in range(3)]

    def mk_waiter(engine):
        seen = {}

        def w(sem, val):
            if seen.get(id(sem), 0) < val:
                engine.wait_ge(sem, val)
                seen[id(sem)] = val

        return w

    with (
        nc.Block() as block,
        nc.semaphore("dmaA") as dmaA,
        nc.semaphore("dmaA2") as dmaA2,
        nc.semaphore("dmaB") as dmaB,
        nc.semaphore("msem") as msem,
        nc.semaphore("pes") as pes,
        nc.semaphore("dves") as dves,
        nc.semaphore("evA") as evA,
        nc.semaphore("evB") as evB,
        nc.semaphore("odma") as odma,
    ):

        @block.sync
        def _(sync):
            w = mk_waiter(sync)
            # TBA layout: [A (0:128) | TB (128:1152)]
            sync.dma_start(
                out=TBA_s.ap()[:, 0:640], in_=TBA_d[:, 0:640]
            ).then_inc(dmaA, 16)
            sync.dma_start(
                out=TBA_s.ap()[:, 640 : N + 128], in_=TBA_d[:, 640 : N + 128]
            ).then_inc(dmaA2, 16)
            w(dves, 2)
            sync.dma_start(out=u_d[:], in_=K_s.ap()).then_inc(odma, 16)
            for g in range(3):
                w(evA, 2 * (g + 1))
                w(evB, 2 * (g + 1))
                sync.dma_start(
                    out=o_d[g][:],
                    in_=o_s.ap()[:, GSPAN[g][0] : GSPAN[g][1]],
                ).then_inc(odma, 16)

        @block.gpsimd
        def _(gp):
            gp.memset(K_s.ap(), 0.0).then_inc(msem, 1)

        @block.tensor
        def _(tensor):
            w = mk_waiter(tensor)
            w(dmaA, 16)
            tensor.matmul(
                pv.ap()[:, 0:512],
                TBA_s.ap()[:, 0:128],
                TBA_s.ap()[:, 128:640],
                start=True,
                stop=True,
            ).then_inc(pes, 1)
            w(dmaA2, 16)
            tensor.matmul(
                pv.ap()[:, 512:1024],
                TBA_s.ap()[:, 0:128],
                TBA_s.ap()[:, 640:1152],
                start=True,
                stop=True,
            ).then_inc(pes, 1)
            # blocks 0-6 use pg slots 0-6 fresh; b7/b8 take the pv halves
            # freed by the predicated copies (already waited via dves>=2);
            # b9/b10/b11 reuse slots 1,2,3 whose evacs finish much earlier.
            for k, (m, n) in enumerate(BLOCKS):
                w(dves, 1 if n == 0 else 2)
                if k == 6:
                    w(evA, 1)  # slot 0 freed by b0's evac
                elif k == 9:
                    w(evB, 1)  # slot 1 freed by b1's evac
                elif k == 10:
                    w(evA, 2)  # slot 2 freed by b2's evac
                elif k == 11:
                    w(evB, 2)  # slot 3 freed by b3's evac
                lo = LOS[k]
                dst = SLOT_AP(pg, pv, k, lo)
                tensor.matmul(
                    dst,
                    K_s.ap()[:, 128 * m : 128 * m + 128],
                    K_s.ap()[:, 512 * n + lo : 512 * n + 512],
                    start=True,
                    stop=True,
                ).then_inc(pes, 1)

        @block.scalar
        def _(scalar):
            w = mk_waiter(scalar)
            scalar.dma_start(out=MS_s.ap(), in_=MS_d[:]).then_inc(dmaB, 16)
            # preload the Copy activation table while DMAs run
            scalar.copy(scr.ap()[:, 0:1], scr.ap()[:, 1:2])
            for k in (0, 2, 4, 6, 8, 11):
                w(pes, 2 + k + 1)
                lo = LOS[k]
                src = SLOT_AP(pg, pv, k, lo)
                scalar.copy(
                    o_s.ap()[:, OFF[k] : OFF[k] + WID[k]], src
                ).then_inc(evA, 1)

        @block.vector
        def _(vector):
            w = mk_waiter(vector)
            w(msem, 1)
            w(dmaB, 16)
            w(pes, 1)
            vector.copy_predicated(
                K_s.ap()[:, 0:512], MS_s.ap()[:, 0:512], pv.ap()[:, 0:512]
            ).then_inc(dves, 1)
            w(pes, 2)
            vector.copy_predicated(
                K_s.ap()[:, 512:1024], MS_s.ap()[:, 512:1024], pv.ap()[:, 512:1024]
            ).then_inc(dves, 1)
            for k in (1, 3, 5, 7, 9, 10):
                w(pes, 2 + k + 1)
                lo = LOS[k]
                src = SLOT_AP(pg, pv, k, lo)
                vector.tensor_copy(
                    o_s.ap()[:, OFF[k] : OFF[k] + WID[k]], src
                ).then_inc(evB, 1)

    return nc


def _cheb_basis(vals, Dp):
    z = 2.0 * vals - 1.0
    B = np.zeros((len(vals), Dp), np.float64)
    B[:, 0] = 1.0
    if Dp > 1:
        B[:, 1] = z
    for k in range(2, Dp):
        B[:, k] = 2 * z * B[:, k - 1] - B[:, k - 2]
    return B


def _fit_coeffs(W1, b1, W2, b2, W3, b3):
    # 2D Chebyshev-interpolation coefficients of the full MLP scalar output
    # v(s, t) on [0,1]^2, via tensor Chebyshev grid + DCT.
    G = GFIT
    k = np.arange(G)
    t = np.cos((2 * k + 1) * np.pi / (2 * G))
    s01 = (t + 1.0) / 2.0
    S, T = np.meshgrid(s01, s01, indexing="ij")
    u = (
        W1[:, 0][:, None, None] * S[None]
        + W1[:, 1][:, None, None] * T[None]
        + b1[:, None, None]
    )
    h2 = np.tensordot(W2, np.tanh(u), axes=(1, 0)) + b2[:, None, None]
    F = np.tensordot(W3[0], np.maximum(h2, 0.0), axes=(0, 0)) + b3[0]
    try:
        from scipy.fft import dct
    except ImportError:  # pragma: no cover
        from scipy.fftpack import dct
    C = dct(dct(F, type=2, axis=-1), type=2, axis=-2) / (G * G)
    C[0, :] /= 2.0
    C[:, 0] /= 2.0
    return C[:DV, :DV]


def kernel(x, W1, b1, W2, b2, W3, b3, sigma, _trace=False):
    from concourse.bass_utils import run_bass_kernel_spmd
    import ml_dtypes

    bf16 = ml_dtypes.bfloat16

    x = np.asarray(x, np.float64).reshape(N)
    W1 = np.asarray(W1, np.float64)
    b1 = np.asarray(b1, np.float64).reshape(128)
    W2 = np.asarray(W2, np.float64)
    b2 = np.asarray(b2, np.float64).reshape(32)
    W3 = np.asarray(W3, np.float64).reshape(1, 32)
    b3 = np.asarray(b3, np.float64).reshape(1)
    sig = float(np.asarray(sigma, np.float64).reshape(-1)[0])

    if "nc" not in _BUILD_CACHE:
        _BUILD_CACHE["nc"] = _build()
    nc = _BUILD_CACHE["nc"]

    C = _fit_coeffs(W1, b1, W2, b2, W3, b3)  # [DV, DV]
    Sb = _cheb_basis(x, DV)  # [N, DV]

    jcols = np.arange(N)
    in_maps = []
    ips = []
    for c in range(NCORES):
        ip = 8 * np.arange(128) + c
        ips.append(ip)
        TBA = np.zeros((DV, N + 128), bf16)
        TBA[:, :128] = (C.T @ Sb[ip].T).astype(bf16)
        TBA[:, 128:] = Sb.T.astype(bf16)
        MS = (jcols[None, :] > ip[:, None]).astype(np.uint8)
        in_maps.append({"TBA": TBA, "MS": MS})

    res = run_bass_kernel_spmd(
        nc, in_maps, core_ids=list(range(NCORES)), trace=_trace
    )
    global LAST_RESULT
    LAST_RESULT = res

    # Sum bf16 partial gram blocks (M^T M) across cores, mirror, then add the
    # identity-row cross terms: out = sigma^2 (M^T M + U + U^T + I).
    P = np.zeros((N, N), np.float32)
    for k, (m, n) in enumerate(BLOCKS):
        g = k // 4
        lo = LOS[k]
        st = OFF[k] - GSPAN[g][0]
        acc = np.zeros((128, WID[k]), np.float32)
        for c in range(NCORES):
            acc += res.results[c][f"o{g}"][:, st : st + WID[k]].astype(
                np.float32
            )
        P[128 * m : 128 * (m + 1), 512 * n + lo : 512 * (n + 1)] = acc
    # mirror everything left of each row-block's diagonal start
    for m in range(1, 8):
        P[128 * m : 128 * (m + 1), 0 : 128 * m] = P[
            0 : 128 * m, 128 * m : 128 * (m + 1)
        ].T
    U = np.zeros((N, N), np.float32)
    for c in range(NCORES):
        U[ips[c], :] = res.results[c]["u"].astype(np.float32)
    P += U
    P += U.T
    P[np.arange(N), np.arange(N)] += 1.0
    return (np.float32(sig) * np.float32(sig)) * P
